# revision 12
# baseline (speedup 1.0000x reference)
"""Weighted-Dice-loss (nn_DiceLoss) Trainium2 Bass kernel, v2.

Full inputs: pred [64,1,512,512] f32, mask [64,1,512,512] f32.
Output: scalar f32 = mean over images of 1 - (2*inter+0.5)/(union+0.5) with
  weit  = 1 + 5*|boxavg31(mask) - mask|
  inter = sum(sigmoid(pred)*mask*weit),  union = sum((sigmoid(pred)+mask)*weit)

Sharding: pure data parallel, 8 images per NeuronCore; tiny per-image partial
sums come back per-core and the final reduction happens on the host.

v2 engine split (all four compute engines loaded; inputs cast to bf16 on the
host, halving HBM traffic):
  DVE   per image: e = m[t+31]-m[t] window-difference (bf16 TT, 2x), the
        sliding 31-box scan as a 1-ALU running sum over e (add/bypass),
        a1 = 1 + a (tensor_scalar, 4x), tt = s2*p (TT, 2x), and the diag
        extraction of the C-PSUM (tensor_mask_reduce).
  PE    band matmuls for the H-box with the -960*I 'negi' diagonal
        (bf16(-961) is inexact; -960 shifts the weit m-coefficient by 0.1%,
        far inside the 2e-2 gate), one-hot-stationary reductions of tt and
        s2 accumulated across all 8 images in two PSUM banks, and a
        diag-trick product for C = sum(p*a): 16 [128]x[128] block matmuls
        p_blk^T @ a_blk accumulated into one [128,128] PSUM whose diagonal
        holds per-column partial sums.
  ACT   sigmoid(pred) (accum -> sum p) and abs(d)*(5/961) -> a (PSUM->SBUF).
  Pool  s2 = a1*m via gpsimd tensor_tensor (the otherwise-idle engine).

Host: per-image inter/union from the partials, wdiss, mean over 64 images.
"""

import numpy as np
import ml_dtypes
from contextlib import ExitStack

import concourse.tile as tile
from concourse import bacc, mybir
from concourse.bass_utils import run_bass_kernel_spmd

N_CORES = 8
B_PER_CORE = 8
H = W = 512
PB = 128          # SBUF partitions
NJ = H // PB      # 4 row-blocks per image
PADW = 16 + W + 16  # 544: padded row for the scan
KHALF = 15        # box radius
KK = 961.0        # 31*31
NEGI_DIAG = -960.0  # bf16-exact stand-in for -961 (see module docstring)

f32 = mybir.dt.float32
bf16 = mybir.dt.bfloat16
Alu = mybir.AluOpType
Act = mybir.ActivationFunctionType

# engine-placement toggles
SCAN_1ALU = True      # e = TT-sub then add/bypass scan (vs 2-ALU direct scan)
S2_ON_GPSIMD = True   # s2 = a1*m on Pool (vs DVE)
C_VIA_DIAG = True     # C = sum(p*a) via PE diag-trick (vs DVE STT)


def _host_constants():
    r = np.arange(PB)[:, None]
    c = np.arange(PB)[None, :]
    cb = np.zeros((PB, 3, PB), dtype=np.float32)
    cb[:, 0, :] = (r - c >= PB - KHALF)
    cb[:, 1, :] = (np.abs(r - c) <= KHALF)
    cb[:, 2, :] = (c - r >= PB - KHALF)
    negi = (NEGI_DIAG * np.eye(PB, dtype=np.float32)).astype(ml_dtypes.bfloat16)
    onehot = np.zeros((PB, B_PER_CORE, PB), dtype=np.float32)
    for b in range(B_PER_CORE):
        onehot[:, b, b] = 1.0
    iota = np.stack([np.arange(PB, dtype=np.float32),
                     np.arange(1, PB + 1, dtype=np.float32)], axis=1)
    return (cb.astype(ml_dtypes.bfloat16), negi,
            onehot.astype(ml_dtypes.bfloat16), iota)


def _build():
    nc = bacc.Bacc("TRN2", target_bir_lowering=False, debug=False,
                   num_devices=N_CORES)
    pred_d = nc.dram_tensor("pred", [B_PER_CORE, H, W], bf16, kind="ExternalInput")
    mq_d = nc.dram_tensor("mq", [B_PER_CORE, H, W], bf16, kind="ExternalInput")
    band_d = nc.dram_tensor("band", [PB, 3, PB], bf16, kind="ExternalInput")
    negi_d = nc.dram_tensor("negi", [PB, PB], bf16, kind="ExternalInput")
    oh_d = nc.dram_tensor("onehot", [PB, B_PER_CORE, PB], bf16, kind="ExternalInput")
    iota_d = nc.dram_tensor("iota", [PB, 2], f32, kind="ExternalInput")
    acc_d = nc.dram_tensor("acc", [PB, 32], f32, kind="ExternalOutput")
    red_d = nc.dram_tensor("red", [B_PER_CORE, 2 * W], f32, kind="ExternalOutput")
    dg_d = nc.dram_tensor("dg", [PB, B_PER_CORE * PB], bf16, kind="ExternalOutput")

    pred_r = pred_d.ap().rearrange("b (j p) w -> b p j w", p=PB)
    mq_r = mq_d.ap().rearrange("b (j p) w -> b p j w", p=PB)

    SCANL = (NJ - 1) * PADW + W  # 2144

    with tile.TileContext(nc) as tc:
        with ExitStack() as ctx:
            cpool = ctx.enter_context(tc.tile_pool(name="cpool", bufs=1))
            ppool = ctx.enter_context(tc.tile_pool(name="ppool", bufs=3))
            epool = ctx.enter_context(tc.tile_pool(name="epool", bufs=2))
            upool = ctx.enter_context(tc.tile_pool(name="upool", bufs=3))
            apool = ctx.enter_context(tc.tile_pool(name="apool", bufs=3))
            a1pool = ctx.enter_context(tc.tile_pool(name="a1pool", bufs=3))
            sigpool = ctx.enter_context(tc.tile_pool(name="sigpool", bufs=3))
            s2pool = ctx.enter_context(tc.tile_pool(name="s2pool", bufs=3))
            ttpool = ctx.enter_context(tc.tile_pool(name="ttpool", bufs=3))
            scrpool = ctx.enter_context(tc.tile_pool(name="scrpool", bufs=2))
            pspool = ctx.enter_context(tc.tile_pool(name="pspool", bufs=2, space="PSUM"))
            dgpool = ctx.enter_context(tc.tile_pool(name="dgpool", bufs=2, space="PSUM"))
            redpool = ctx.enter_context(tc.tile_pool(name="redpool", bufs=1, space="PSUM"))

            cb = cpool.tile([PB, 3, PB], bf16, name="cb")
            nc.sync.dma_start(cb[:], band_d.ap())
            negi = cpool.tile([PB, PB], bf16, name="negi")
            nc.sync.dma_start(negi[:], negi_d.ap())
            oh = cpool.tile([PB, B_PER_CORE, PB], bf16, name="oh")
            nc.sync.dma_start(oh[:], oh_d.ap())
            iota = cpool.tile([PB, 2], f32, name="iota")
            nc.sync.dma_start(iota[:], iota_d.ap())

            acc = cpool.tile([PB, 32], f32, name="acc")
            redsb = cpool.tile([B_PER_CORE, 2 * W], f32, name="redsb")
            dgall = cpool.tile([PB, B_PER_CORE * PB], bf16, name="dgall")

            # persistent mask tiles (4-deep rotation); pad columns zeroed once
            mp_tiles = []
            for i in range(4):
                mpt = cpool.tile([PB, 32 + NJ * PADW], bf16, name=f"mpt{i}")
                nc.vector.memset(mpt[:, 0:32], 0.0)
                m3 = mpt[:, 32:].rearrange("p (j w) -> p j w", j=NJ)
                nc.vector.memset(m3[:, :, 0:16], 0.0)
                nc.vector.memset(m3[:, :, 16 + W:PADW], 0.0)
                mp_tiles.append(mpt)

            # cross-image one-hot reduction PSUMs (accumulated over the whole
            # kernel): row b holds image b's column sums.
            red_tt = redpool.tile([PB, W], f32, name="red_tt")
            red_s2 = redpool.tile([PB, W], f32, name="red_s2")

            stash = {}
            for b in range(B_PER_CORE + 1):
                if b < B_PER_CORE:
                    # ---- stage A: load, W-box, H-box matmuls, ACT ----
                    mp = mp_tiles[b % 4]
                    mp3 = mp[:, 32:].rearrange("p (j w) -> p j w", j=NJ)
                    if b == 0:
                        nc.sync.dma_start(mp3[:, 0:2, 16:16 + W], mq_r[b][:, 0:2])
                        nc.sync.dma_start(mp3[:, 2:4, 16:16 + W], mq_r[b][:, 2:4])
                    else:
                        nc.sync.dma_start(mp3[:, :, 16:16 + W], mq_r[b])

                    pt = ppool.tile([PB, NJ * W], bf16, name="pt")
                    pt3 = pt.rearrange("p (j w) -> p j w", j=NJ)
                    nc.sync.dma_start(pt3[:], pred_r[b])

                    # W-axis 31-box sliding window.  The >=31 zero-pad columns
                    # between row-blocks keep windows from mixing rows and
                    # give count_include_pad edge clamping for free.
                    uext = upool.tile([PB, 31 + SCANL], bf16, name="uext")
                    u = uext[:, 31:]
                    ranges = ([(-31, SCANL)] if 0 < b < B_PER_CORE - 1
                              else [(-31, 2 * PADW - 16), (2 * PADW - 16, SCANL)])
                    if SCAN_1ALU:
                        # e_t = m[t+31] - m[t] (2x TT), then a 1-ALU running
                        # sum: state = (e + state) [bypass d1].
                        eext = epool.tile([PB, 31 + SCANL], bf16, name="eext")
                        nc.vector.tensor_tensor(
                            eext[:], mp[:, 63 - 31:63 + SCANL],
                            mp[:, 32 - 31:32 + SCANL], Alu.subtract)
                        for t0, t1 in ranges:
                            nc.vector.tensor_tensor_scan(
                                uext[:, 31 + t0:31 + t1],
                                eext[:, 31 + t0:31 + t1],
                                eext[:, 31 + t0:31 + t1], 0.0,
                                Alu.add, Alu.bypass)
                    else:
                        for t0, t1 in ranges:
                            nc.vector.tensor_tensor_scan(
                                uext[:, 31 + t0:31 + t1], mp[:, 63 + t0:63 + t1],
                                mp[:, 32 + t0:32 + t1], 0.0,
                                Alu.add, Alu.subtract)

                    # H-box band matmuls + folded -960*I diagonal -> d PSUM;
                    # ACT abs -> a;  a1 = 1 + a (4x tensor_scalar).
                    a = apool.tile([PB, NJ * W], bf16, name="a")
                    for hh in range(2):
                        psh = pspool.tile([PB, 2 * W], f32, name="psh")
                        ps3 = psh.rearrange("p (j w) -> p j w", j=2)
                        for jj in range(2):
                            j = 2 * hh + jj
                            nc.tensor.matmul(
                                ps3[:, jj, :], cb[:, 1, :],
                                u[:, j * PADW:j * PADW + W],
                                start=True, stop=False)
                            if j >= 1:
                                nc.tensor.matmul(
                                    ps3[:, jj, :], cb[:, 0, :],
                                    u[:, (j - 1) * PADW:(j - 1) * PADW + W],
                                    start=False, stop=False)
                            if j <= NJ - 2:
                                nc.tensor.matmul(
                                    ps3[:, jj, :], cb[:, 2, :],
                                    u[:, (j + 1) * PADW:(j + 1) * PADW + W],
                                    start=False, stop=False)
                            nc.tensor.matmul(
                                ps3[:, jj, :], negi[:], mp3[:, j, 16:16 + W],
                                start=False, stop=True)
                        nc.scalar.activation(
                            a[:, hh * 2 * W:(hh + 1) * 2 * W], psh[:],
                            Act.Abs, bias=0.0, scale=5.0 / KK)
                    a1 = a1pool.tile([PB, NJ * W], bf16, name="a1")
                    nc.vector.tensor_scalar(a1[:], a[:], 1.0, None, Alu.add)
                    sg = sigpool.tile([PB, NJ * W], bf16, name="sg")
                    nc.scalar.activation(sg[:], pt[:], Act.Sigmoid,
                                         accum_out=acc[:, 2 * b + 1:2 * b + 2])
                    stash[b] = (mp3, a, a1, sg)

                if b >= 1:
                    # ---- stage B: products + reductions for image b-1 ----
                    bb = b - 1
                    mp3p, ap, a1p, sgp = stash.pop(bb)
                    # s2 = (1+a)*m  (Pool engine)
                    s2 = s2pool.tile([PB, NJ * W], bf16, name="s2")
                    eng = nc.gpsimd if S2_ON_GPSIMD else nc.vector
                    eng.tensor_tensor(s2[:], a1p[:], mp3p[:, :, 16:16 + W],
                                      Alu.mult)
                    # tt = s2*p  (p*mask*weit)
                    ttt = ttpool.tile([PB, NJ * W], bf16, name="ttt")
                    nc.vector.tensor_tensor(ttt[:], s2[:], sgp[:], Alu.mult)
                    # one-hot reductions into per-image rows
                    tt3 = ttt.rearrange("p (j w) -> p j w", j=NJ)
                    s23 = s2.rearrange("p (j w) -> p j w", j=NJ)
                    for j in range(NJ):
                        nc.tensor.matmul(red_tt[:], oh[:, bb, :], tt3[:, j, :],
                                         start=(bb == 0 and j == 0),
                                         stop=(bb == B_PER_CORE - 1 and j == NJ - 1))
                    for j in range(NJ):
                        nc.tensor.matmul(red_s2[:], oh[:, bb, :], s23[:, j, :],
                                         start=(bb == 0 and j == 0),
                                         stop=(bb == B_PER_CORE - 1 and j == NJ - 1))
                    if C_VIA_DIAG:
                        # C = sum(p*a) via block-diag trick: host reads the
                        # diagonal of the accumulated [128,128] block product.
                        dg = dgpool.tile([PB, PB], f32, name="dg")
                        for kb in range(4 * NJ):
                            nc.tensor.matmul(
                                dg[:], sgp[:, kb * PB:(kb + 1) * PB],
                                ap[:, kb * PB:(kb + 1) * PB],
                                start=(kb == 0), stop=(kb == 4 * NJ - 1))
                        nc.scalar.copy(dgall[:, bb * PB:(bb + 1) * PB], dg[:])
                    else:
                        pa = ttpool.tile([PB, NJ * W], bf16, name="pa", tag="pa")
                        nc.vector.scalar_tensor_tensor(
                            pa[:], sgp[:], 1.0, ap[:], Alu.mult, Alu.mult,
                            accum_out=acc[:, 16 + bb:17 + bb])

            nc.scalar.copy(redsb[:, 0:W], red_tt[0:B_PER_CORE, :])
            nc.scalar.copy(redsb[:, W:2 * W], red_s2[0:B_PER_CORE, :])
            nc.sync.dma_start(acc_d.ap(), acc[:])
            nc.sync.dma_start(red_d.ap(), redsb[:])
            nc.sync.dma_start(dg_d.ap(), dgall[:])

    nc.compile()
    return nc


_NC = None


def _get_nc():
    global _NC
    if _NC is None:
        _NC = _build()
    return _NC


def _in_maps(pred, mask):
    band, negi, onehot, iota = _host_constants()
    pred = np.asarray(pred, dtype=np.float32).reshape(64, H, W)
    mask = np.asarray(mask, dtype=np.float32).reshape(64, H, W)
    pred16 = pred.astype(ml_dtypes.bfloat16)
    mq = mask.astype(ml_dtypes.bfloat16)
    ims = []
    for c in range(N_CORES):
        sl = slice(c * B_PER_CORE, (c + 1) * B_PER_CORE)
        ims.append({
            "pred": np.ascontiguousarray(pred16[sl]),
            "mq": np.ascontiguousarray(mq[sl]),
            "band": band,
            "negi": negi,
            "onehot": onehot,
            "iota": iota,
        })
    return ims


def _host_reduce(results):
    """per-core acc [128, 32] + red [16, 512] f32 -> final scalar loss."""
    wd = []
    for r in results:
        a = r["acc"].astype(np.float64)
        rd = r["red"].astype(np.float64)
        dgr = r["dg"].astype(np.float64)
        for b in range(B_PER_CORE):
            psum = a[:, 2 * b + 1].sum()          # sum(p)
            if C_VIA_DIAG:
                pa = np.diagonal(dgr[:, b * PB:(b + 1) * PB]).sum()
            else:
                pa = a[:, 16 + b].sum()           # sum(p*(weit-1))
            inter = rd[b, 0:W].sum()              # sum(p*mask*weit)
            mw = rd[b, W:2 * W].sum()             # sum(mask*weit)
            union = psum + pa + mw
            wd.append(1.0 - (2.0 * inter + 0.5) / (union + 0.5))
    return np.array(np.mean(wd), dtype=np.float32)


def kernel(pred, mask):
    nc = _get_nc()
    res = run_bass_kernel_spmd(nc, _in_maps(pred, mask),
                               core_ids=list(range(N_CORES)))
    return _host_reduce(res.results)
